# revision 11
# baseline (speedup 1.0000x reference)
"""Trainium2 Bass kernel for nn_EpistemicMemory (retrieval_knn).

Reference computation:
    qn = l2norm(queries.reshape(BT, dk)); kn = l2norm(keys)
    scores = qn @ kn.T                       # [BT, N]
    topk_scores, topk_idx = top_k(scores, 32)
    attn = softmax(topk_scores / 0.1)
    out = attn @ values[topk_idx]            # [BT, dv]
    returns (out, attn, topk_idx)

Device strategy (8 cores):
  Phase 1 (keys sharded along N): each core holds knT shard [dk, N/8] in
    SBUF and computes scores for all BT queries with fp32r matmuls,
    512-column blocks. Per block, DVE max8 extracts the block top-8 and
    max_index their positions (global top-32 of a query lands >8-deep in a
    512-block with probability ~1e-7, so per-block top-8 is a safe
    candidate set). Candidate (value, global-key-index) pairs are staged
    to an internal DRAM buffer.
  Phase 2: single AllToAll moves each query-shard's candidates to its
    owning core (rows q of rank r's buffer -> core q//256).
  Phase 3 (queries sharded): merge 8*NBLK*8 candidates per query with
    max8/match_replace rounds -> exact top-32 + slots via max_index;
    gather winner key indices from the A2A buffer and winner value rows
    from the full `values` tensor with indirect DMA; softmax on ACT;
    weighted sum on DVE. Outputs are the core's 256-query slice.

kernel() is self-contained: normalization/transposition of inputs is host
prep; all scoring/selection/aggregation runs on the NeuronCores.
"""

from dataclasses import dataclass

import numpy as np

import bass_rust
import concourse.bass as bass
import concourse.mybir as mybir
from concourse.tile import TileContext
from concourse.bass_utils import run_bass_kernel_spmd

def split_ctrl_waits(nc, max_waits: int = 1):
    """Workaround: this walrus build rejects instructions carrying more than
    one sync-wait ("Too many sync wait commands" in setupSyncWait). Move extra
    waits onto NoOps inserted immediately before, on the same engine stream.
    Apply only before hardware compile (CoreSim rejects the bare NoOps)."""
    n_fixed = 0
    for fn in nc.m.functions:
        for bb in fn.blocks:
            insts = bb.instructions
            i = 0
            while i < len(insts):
                inst = insts[i]
                si = inst.sync_info
                if (
                    si is not None
                    and si.on_wait
                    and len(si.on_wait) > max_waits
                ):
                    waits = list(si.on_wait)
                    keep = waits[:max_waits]
                    extra = waits[max_waits:]
                    new_nops = []
                    for j, w in enumerate(extra):
                        nop = mybir.InstNoOp(
                            name=f"{inst.name}-waitsplit-{j}", ins=[], outs=[])
                        nop.engine = inst.engine
                        nop.sync_info = bass_rust.SyncInfo(on_wait=[w], on_update=[])
                        new_nops.append(nop)
                    inst.sync_info = bass_rust.SyncInfo(
                        on_wait=keep, on_update=list(si.on_update or []))
                    for j, nop in enumerate(new_nops):
                        insts.insert(i + j, nop)
                    i += len(new_nops)
                    n_fixed += 1
                i += 1
    return n_fixed

import os
F32 = mybir.dt.float32
F32R = mybir.dt.float32 if os.environ.get("MM_F32") else mybir.dt.float32r
U32 = mybir.dt.uint32
I32 = mybir.dt.int32
EPS = 1e-12


@dataclass(frozen=True)
class Cfg:
    BT: int = 2048      # total queries
    DK: int = 512       # key dim
    DV: int = 512       # value dim
    N: int = 65536      # memory size
    W: int = 8          # cores
    BLK: int = 512      # score block (psum bank) width
    K: int = 32         # top-k

    @property
    def NS(self):       # keys per core
        return self.N // self.W

    @property
    def NBLK(self):     # score blocks per core
        return self.NS // self.BLK

    @property
    def CAND(self):     # candidates per rank per query
        return 8 * self.NBLK

    @property
    def CALL(self):     # merged candidates per query
        return self.W * self.CAND

    @property
    def QS(self):       # queries per core in phase 3
        return self.BT // self.W

    @property
    def QT(self):       # phase-1 query tiles
        return self.BT // 128

    @property
    def QTL(self):      # phase-3 local query tiles
        return self.QS // 128

    @property
    def DCH(self):      # contraction chunks
        return self.DK // 128


FULL = Cfg()


def build_kernel(cfg: Cfg = FULL):
    c = cfg
    assert c.CALL <= 16384 and c.CAND >= 8 and c.QS % 128 == 0
    nc = bass.Bass("TRN2", target_bir_lowering=False, debug=False,
                   num_devices=c.W)

    qnT = nc.declare_dram_parameter("qnT", [c.DK, c.BT], F32R, isOutput=False)
    knT = nc.declare_dram_parameter("knT", [c.DK, c.NS], F32R, isOutput=False)
    values = nc.declare_dram_parameter("values", [c.N, c.DV], F32, isOutput=False)
    # per-core constants: rank_base[p,0] = rank*NS ; qoff[p,t] = flat gki
    # element offset of (local query 128*t+p)'s row in the a2a buffer
    rank_base = nc.declare_dram_parameter("rank_base", [128, 1], F32, isOutput=False)
    qoff = nc.declare_dram_parameter("qoff", [128, max(c.QTL, 1)], U32, isOutput=False)

    agg_out = nc.declare_dram_parameter("agg_out", [c.QS, c.DV], F32, isOutput=True)
    attn_out = nc.declare_dram_parameter("attn_out", [c.QS, c.K], F32, isOutput=True)
    idx_out = nc.declare_dram_parameter("idx_out", [c.QS, c.K], I32, isOutput=True)

    with TileContext(nc) as tc:
        with (
            tc.tile_pool(name="const", bufs=1) as constp,
            tc.tile_pool(name="dram", bufs=1, space="DRAM") as dp,
        ):
            # ---- constants ----
            rb = constp.tile([128, 1], F32, name="rb")
            nc.sync.dma_start(out=rb[:], in_=rank_base[:, :])
            qoff_t = constp.tile([128, max(c.QTL, 1)], U32, name="qoff_t")
            nc.sync.dma_start(out=qoff_t[:], in_=qoff[:, :])
            # base_tile[p, blk*8+j] = blk*BLK
            base_tile = constp.tile([128, c.CAND], F32, name="base_tile")
            for blk in range(c.NBLK):
                nc.vector.memset(base_tile[:, blk * 8:(blk + 1) * 8],
                                 float(blk * c.BLK))

            # ---- phase 1 scope: keys resident in SBUF ----
            p1 = tc.tile_pool(name="keys", bufs=1)
            keysp = p1.__enter__()
            p1q = tc.tile_pool(name="qtiles", bufs=2)
            qp = p1q.__enter__()
            p1w = tc.tile_pool(name="work", bufs=2)
            wp = p1w.__enter__()
            p1p = tc.tile_pool(name="psum", bufs=4, space="PSUM")
            pp = p1p.__enter__()
            knt = []
            for d in range(c.DCH):
                kt = keysp.tile([128, c.NS], F32R, name=f"knt{d}", tag=f"knt{d}")
                nc.sync.dma_start(out=kt[:], in_=knT[d * 128:(d + 1) * 128, :])
                knt.append(kt)

            # ---- A2A buffers ----
            a2a_in = dp.tile([c.BT, 2 * c.CAND], F32, name="a2a_in")
            a2a_out = dp.tile([c.BT, 2 * c.CAND], F32, name="a2a_out")

            # ---- phase 1: scores + per-block top-8 ----
            for qt in range(c.QT):
                qtl = []
                for d in range(c.DCH):
                    qd = qp.tile([128, 128], F32R, name=f"q{d}", tag=f"q{d}")
                    nc.sync.dma_start(
                        out=qd[:],
                        in_=qnT[d * 128:(d + 1) * 128, qt * 128:(qt + 1) * 128],
                    )
                    qtl.append(qd)
                cand_val = wp.tile([128, c.CAND], F32, name="cand_val")
                cand_pos = wp.tile([128, c.CAND], U32, name="cand_pos")
                for blk in range(c.NBLK):
                    ps = pp.tile([128, c.BLK], F32, name="ps", space="PSUM")
                    for d in range(c.DCH):
                        nc.tensor.matmul(
                            out=ps[:],
                            lhsT=qtl[d][:],
                            rhs=knt[d][:, blk * c.BLK:(blk + 1) * c.BLK],
                            start=(d == 0),
                            stop=(d == c.DCH - 1),
                        )
                    nc.vector.max(out=cand_val[:, blk * 8:(blk + 1) * 8], in_=ps[:])
                    nc.vector.max_index(
                        cand_pos[:, blk * 8:(blk + 1) * 8],
                        cand_val[:, blk * 8:(blk + 1) * 8],
                        ps[:],
                    )
                # gki = rank_base + blk*BLK + pos
                gki = wp.tile([128, c.CAND], F32, name="gki")
                nc.vector.tensor_copy(out=gki[:], in_=cand_pos[:])
                nc.vector.tensor_tensor(out=gki[:], in0=gki[:], in1=base_tile[:],
                                        op=mybir.AluOpType.add)
                nc.vector.tensor_scalar_add(gki[:], gki[:], rb[:, 0:1])
                nc.sync.dma_start(
                    out=a2a_in[qt * 128:(qt + 1) * 128, 0:c.CAND], in_=cand_val[:])
                nc.sync.dma_start(
                    out=a2a_in[qt * 128:(qt + 1) * 128, c.CAND:2 * c.CAND], in_=gki[:])

            p1p.__exit__(None, None, None)
            p1w.__exit__(None, None, None)
            p1q.__exit__(None, None, None)
            p1.__exit__(None, None, None)

            # ---- phase 2: AllToAll ----
            nc.gpsimd.collective_compute(
                "AllToAll",
                mybir.AluOpType.bypass,
                replica_groups=[list(range(c.W))],
                ins=[a2a_in[:, :]],
                outs=[a2a_out[:, :]],
            )

            # ---- phase 3: merge + outputs ----
            p3w = tc.tile_pool(name="w3", bufs=2)
            wp3 = p3w.__enter__()
            p3v = tc.tile_pool(name="vg", bufs=2)
            vgp = p3v.__enter__()
            # a2a_out rows: s*QS + j = rank s's candidates for local query j
            a2a_v = a2a_out[:, :].rearrange("(s q) c -> q s c", s=c.W)
            for t in range(c.QTL):
                val_t = wp3.tile([128, c.W, c.CAND], F32, name="val_t")
                gki_t = wp3.tile([128, c.W, c.CAND], F32, name="gki_t")
                nc.sync.dma_start(
                    out=val_t[:],
                    in_=a2a_v[t * 128:(t + 1) * 128, :, 0:c.CAND])
                nc.sync.dma_start(
                    out=gki_t[:],
                    in_=a2a_v[t * 128:(t + 1) * 128, :, c.CAND:2 * c.CAND])
                val_f = val_t[:].rearrange("p s c -> p (s c)")
                gki_f = gki_t[:].rearrange("p s c -> p (s c)")

                win = wp3.tile([128, c.K], F32, name="win")
                work = wp3.tile([128, c.CALL], F32, name="work")
                nrounds = c.K // 8
                for r in range(nrounds):
                    src = val_f if r == 0 else work[:]
                    nc.vector.max(out=win[:, r * 8:(r + 1) * 8], in_=src)
                    if r < nrounds - 1:
                        nc.vector.match_replace(
                            out=work[:], in_to_replace=win[:, r * 8:(r + 1) * 8],
                            in_values=src, imm_value=-2.0)
                slots = wp3.tile([128, c.K], U32, name="slots")
                for r in range(nrounds):
                    nc.vector.max_index(
                        slots[:, r * 8:(r + 1) * 8],
                        win[:, r * 8:(r + 1) * 8],
                        val_f,
                    )
                # winner gki via indirect gather from a2a_out:
                # flat elem idx = (s*QS + (t*128+p))*2C + CAND + cc
                #   with slot = s*CAND + cc
                off = wp3.tile([128, c.K], U32, name="off")
                scaled = wp3.tile([128, c.K], U32, name="scaled")
                shift = int(np.log2(c.CAND))
                nc.vector.tensor_scalar(
                    out=scaled[:], in0=slots[:],
                    scalar1=shift, scalar2=None,
                    op0=mybir.AluOpType.logical_shift_right,
                )
                nc.vector.tensor_scalar(
                    out=scaled[:], in0=scaled[:],
                    scalar1=c.QS * 2 * c.CAND, scalar2=None,
                    op0=mybir.AluOpType.mult,
                )
                nc.vector.tensor_scalar(
                    out=off[:], in0=slots[:],
                    scalar1=c.CAND - 1, scalar2=None,
                    op0=mybir.AluOpType.bitwise_and,
                )
                nc.vector.tensor_tensor(out=off[:], in0=off[:], in1=scaled[:],
                                        op=mybir.AluOpType.add)
                nc.vector.tensor_tensor(
                    out=off[:], in0=off[:],
                    in1=qoff_t[:, t:t + 1].to_broadcast([128, c.K]),
                    op=mybir.AluOpType.add)

                gkiw = wp3.tile([128, c.K, 1], F32, name="gkiw")
                a2a_flat = a2a_out[:, :].rearrange("q c -> (q c) ()")
                for k in range(c.K):
                    nc.gpsimd.indirect_dma_start(
                        out=gkiw[:, k, :], out_offset=None,
                        in_=a2a_flat,
                        in_offset=bass.IndirectOffsetOnAxis(
                            ap=off[:, k:k + 1], axis=0),
                    )
                gkiw2 = gkiw[:].rearrange("p k o -> p (k o)")

                # softmax over win / temperature
                bias = wp3.tile([128, 1], F32, name="bias")
                nc.vector.tensor_scalar_mul(bias[:], win[:, 0:1], -10.0)
                attn = wp3.tile([128, c.K], F32, name="attn")
                esum = wp3.tile([128, 1], F32, name="esum")
                nc.scalar.activation(
                    out=attn[:], in_=win[:],
                    func=mybir.ActivationFunctionType.Exp,
                    bias=bias[:, 0:1], scale=10.0,
                    accum_out=esum[:, 0:1],
                )
                rsum = wp3.tile([128, 1], F32, name="rsum")
                nc.vector.reciprocal(rsum[:], esum[:])
                nc.vector.tensor_scalar_mul(attn[:], attn[:], rsum[:, 0:1])

                # gather winner value rows and reduce
                gki_u = wp3.tile([128, c.K], U32, name="gki_u")
                nc.vector.tensor_copy(out=gki_u[:], in_=gkiw2)
                vg = vgp.tile([128, c.K, c.DV], F32, name="vg")
                for k in range(c.K):
                    nc.gpsimd.indirect_dma_start(
                        out=vg[:, k, :], out_offset=None,
                        in_=values[:, :],
                        in_offset=bass.IndirectOffsetOnAxis(
                            ap=gki_u[:, k:k + 1], axis=0),
                    )
                nc.vector.tensor_tensor(
                    out=vg[:], in0=vg[:],
                    in1=attn[:].rearrange("p (k o) -> p k o", o=1)
                        .to_broadcast([128, c.K, c.DV]),
                    op=mybir.AluOpType.mult,
                )
                agg = wp3.tile([128, c.DV], F32, name="agg")
                nc.vector.tensor_reduce(
                    out=agg[:],
                    in_=vg[:].rearrange("p k d -> p d k"),
                    axis=mybir.AxisListType.X,
                    op=mybir.AluOpType.add,
                )

                idxw = wp3.tile([128, c.K], I32, name="idxw")
                nc.vector.tensor_copy(out=idxw[:], in_=gkiw2)

                nc.sync.dma_start(out=agg_out[t * 128:(t + 1) * 128, :], in_=agg[:])
                nc.sync.dma_start(out=attn_out[t * 128:(t + 1) * 128, :], in_=attn[:])
                nc.sync.dma_start(out=idx_out[t * 128:(t + 1) * 128, :], in_=idxw[:])

            p3v.__exit__(None, None, None)
            p3w.__exit__(None, None, None)

    return nc


def _l2n(x):
    n = np.linalg.norm(x, axis=-1, keepdims=True)
    return x / np.maximum(n, EPS)


def make_in_maps(queries, keys, values, cfg: Cfg = FULL):
    c = cfg
    qn = _l2n(queries.reshape(c.BT, c.DK).astype(np.float32))
    kn = _l2n(keys.astype(np.float32))
    qnT = np.ascontiguousarray(qn.T)
    knT = np.ascontiguousarray(kn.T)
    vals = np.ascontiguousarray(values.astype(np.float32))
    p = np.arange(128, dtype=np.uint32)
    qoff = np.stack(
        [(t * 128 + p) * (2 * c.CAND) + c.CAND for t in range(max(c.QTL, 1))],
        axis=1,
    ).astype(np.uint32)
    in_maps = []
    for r in range(c.W):
        in_maps.append({
            "qnT": qnT,
            "knT": np.ascontiguousarray(knT[:, r * c.NS:(r + 1) * c.NS]),
            "values": vals,
            "rank_base": np.full((128, 1), r * c.NS, dtype=np.float32),
            "qoff": qoff,
        })
    return in_maps


_CACHE = {}


def _get_nc(cfg: Cfg = FULL):
    if cfg not in _CACHE:
        nc = build_kernel(cfg)
        split_ctrl_waits(nc)
        _CACHE[cfg] = nc
    return _CACHE[cfg]


def kernel(queries, keys, values):
    c = FULL
    B, T, _ = queries.shape
    nc = _get_nc(c)
    in_maps = make_in_maps(queries, keys, values, c)
    res = run_bass_kernel_spmd(nc, in_maps, core_ids=list(range(c.W)))
    agg = np.concatenate([res.results[r]["agg_out"] for r in range(c.W)], axis=0)
    attn = np.concatenate([res.results[r]["attn_out"] for r in range(c.W)], axis=0)
    idx = np.concatenate([res.results[r]["idx_out"] for r in range(c.W)], axis=0)
    return (
        agg.reshape(B, T, c.DV),
        attn.reshape(B, T, c.K),
        idx.reshape(B, T, c.K).astype(np.int32),
    )
